# revision 22
# baseline (speedup 1.0000x reference)
"""Trainium2 Bass kernel for EnhancedMambaStateSpace.

Full inputs in, full output out. Data-parallel over batch across 8 cores
(2 batch rows per core); SSM params replicated and pre-folded on host.

Math (per batch row b):
  xc = depthwise_conv1d(x, conv_w, pad=1) + conv_b
  sel = softplus(xc @ sel_W.T + sel_b + selection_bias)
  delta = softplus(xc @ delta_W.T + delta_b)
  A = -exp(A_log); Ad = exp(delta * A)
  Bx = (Ad - 1)/(A + 1e-8) * sel * (xc @ Bm.T)
  s_t = Ad_t * s_{t-1} + Bx_t  (scan over L, keep last)
  y = s_L @ Cm.T + xc[:, -1] @ Dm.T

Device layout: tokens on the free dim, d/n on partitions. x is transposed
on-chip with PE identity-matmuls; the conv runs as three shifted
scale-accumulate ops (1 ACT + 2 DVE) into a whole-sequence xc buffer; the
recurrence is a native DVE tensor_tensor_scan chained across 512-token
chunks, batch-packed [b0|b1] on 128 partitions.
"""

from contextlib import ExitStack

import numpy as np

import concourse.bacc as bacc
import concourse.bass as bass
import concourse.tile as tile
from concourse import mybir
from concourse.bass_utils import run_bass_kernel_spmd

B, L, D, N, O = 16, 4096, 256, 64, 256
P = 128          # partitions
CH = 512         # tokens per chunk
NCH = L // CH    # 8 chunks
BPC = 2          # batch rows per core
NCORES = 8

FP = mybir.dt.float32
FPR = mybir.dt.float32r
XDT = mybir.dt.float16
AOP = mybir.AluOpType

_ONE_TABLE = "natural_log_exp_and_others"


def _patch_act_tables():
    """Keep Exp/Ln/Copy resolvable only via one ACT table so the
    act-table-load pass never thrashes between tables (1283ns per load)."""
    import concourse.hw_specs as hw_specs
    import concourse.bacc as _bacc
    if getattr(_bacc, "_act_tables_patched", False):
        return
    orig = hw_specs.get_activation_tables

    def patched(module_arch):
        tabs = orig(module_arch)
        drop = {mybir.ActivationFunctionType.Exp,
                mybir.ActivationFunctionType.Ln,
                mybir.ActivationFunctionType.Copy}
        out = {}
        for name, funcs in tabs.items():
            if name == _ONE_TABLE:
                out[name] = funcs
            else:
                out[name] = funcs - drop
        return out

    _bacc.get_activation_tables = patched
    _bacc._act_tables_patched = True


def _build_program(proj_dtype=FPR):
    _patch_act_tables()
    nc = bacc.Bacc("TRN2", target_bir_lowering=False, debug=False)

    xs = nc.dram_tensor("xs", [BPC, L, D], FP, kind="ExternalInput").ap()
    # 3-tap conv folded into projection weights: [K=d, h, tap, 192]
    wk = nc.dram_tensor("wk", [P, 2, 3, 3 * N], XDT, kind="ExternalInput").ap()
    pcols = nc.dram_tensor("pcols", [P, 4], FP, kind="ExternalInput").ap()
    cmT = nc.dram_tensor("cmT", [P, 2 * O], FP, kind="ExternalInput").ap()
    # Dm with conv folded: [K=d, h, tap, O]
    dmT = nc.dram_tensor("dmT", [P, 2, 2, O], XDT, kind="ExternalInput").ap()
    ybias = nc.dram_tensor("ybias", [1, 2 * O], FP, kind="ExternalInput").ap()
    ident = nc.dram_tensor("ident", [P, P], XDT, kind="ExternalInput").ap()
    y = nc.dram_tensor("y", [1, 2 * O], FP, kind="ExternalOutput").ap()

    with tile.TileContext(nc) as tc, ExitStack() as ctx:
        consts = ctx.enter_context(tc.tile_pool(name="consts", bufs=1))
        xtp = ctx.enter_context(tc.tile_pool(name="xtp", bufs=1))
        xn = ctx.enter_context(tc.tile_pool(name="xn", bufs=4))
        nsb = ctx.enter_context(tc.tile_pool(name="nsb", bufs=6))
        scanp = ctx.enter_context(tc.tile_pool(name="scanp", bufs=3))
        psum = ctx.enter_context(tc.tile_pool(name="psum", bufs=1, space="PSUM"))

        wk_sb = consts.tile([P, 2, 3, 3 * N], XDT, tag="wk")
        pcols_sb = consts.tile([P, 4], FP, tag="pcols")
        cmT_sb = consts.tile([P, 2 * O], FP, tag="cmT")
        dmT_sb = consts.tile([P, 2, 2, O], XDT, tag="dmT")
        ybias_sb = consts.tile([1, 2 * O], FP, tag="ybias")
        ident_sb = consts.tile([P, P], XDT, tag="ident")
        nc.sync.dma_start(out=wk_sb, in_=wk)
        nc.sync.dma_start(out=pcols_sb, in_=pcols)
        nc.sync.dma_start(out=cmT_sb, in_=cmT)
        nc.sync.dma_start(out=dmT_sb, in_=dmT)
        nc.sync.dma_start(out=ybias_sb, in_=ybias)
        nc.sync.dma_start(out=ident_sb, in_=ident)

        # raw transposed x per d-half, both batches: col b*LW + 1 + t = x.T[t]
        # cols 0 and L+1 are zero pads (x[-1] = x[L] = 0 for the conv taps).
        LW = L + 4
        xts = [xtp.tile([P, BPC, LW], XDT, tag=f"xts{h}", name=f"xts{h}")
               for h in (0, 1)]
        for h in (0, 1):
            for b in range(BPC):
                nc.vector.memset(xts[h][:, b, 0:1], 0.0)
                nc.vector.memset(xts[h][:, b, L + 1:L + 2], 0.0)

        s_tile = None
        for c in range(NCH + 1):
            if c < NCH:
                t0 = CH * c
                x_nat = [None, None]
                for b in range(BPC):
                    x_nat[b] = xn.tile([P, CH // P, D], XDT, tag=f"xn{b}",
                                       name=f"xn{b}_{c}")
                    nc.gpsimd.dma_start(
                        out=x_nat[b],
                        in_=xs[b, t0:t0 + CH, :].rearrange(
                            "(s p) d -> p s d", p=P),
                    )
                for h in (0, 1):
                    pxt = psum.tile([P, BPC * CH], XDT, tag=f"xt{h}",
                                    name=f"xt{h}_{c}", bufs=2)
                    for b in range(BPC):
                        for s in range(CH // P):
                            nc.tensor.transpose(
                                out=pxt[:, b * CH + s * P:b * CH + (s + 1) * P],
                                in_=x_nat[b][:, s, h * P:(h + 1) * P],
                                identity=ident_sb,
                            )
                    # evict raw xT to SBUF (DVE 2x fp16 copy)
                    nc.vector.tensor_copy(
                        xts[h][:, :, 1 + t0:1 + t0 + CH],
                        pxt.rearrange("p (b t) -> p b t", b=BPC),
                    )
            if c >= 1:
                t0 = CH * (c - 1)
                # psd: [sel|delta] x (b0 cols 0:CH | b1 cols CH:2CH), 2 banks
                psd = psum.tile([P, BPC * CH], FP, tag="sd", name=f"sd_{c}", bufs=1)
                # pP: batch-packed rows (b0 0:64 | b1 64:128), one bank
                pP = psum.tile([P, CH], FP, tag="bm", name=f"bm_{c}", bufs=1)
                for b in range(BPC):
                    nmm = 0
                    for h in (0, 1):
                        for k in (0, 1, 2):
                            rhs = xts[h][:, b, t0 + k:t0 + k + CH]
                            nc.tensor.matmul(
                                out=psd[:, b * CH:(b + 1) * CH],
                                lhsT=wk_sb[:, h, k, 0:P], rhs=rhs,
                                start=(nmm == 0), stop=(nmm == 5))
                            nc.tensor.matmul(
                                out=pP[N * b:N * (b + 1), :],
                                lhsT=wk_sb[:, h, k, P:P + N], rhs=rhs,
                                start=(nmm == 0), stop=(nmm == 5))
                            nmm += 1
                e_sb = nsb.tile([P, BPC * CH], FP, tag="e", name=f"e_{c}")
                l_sb = nsb.tile([P, BPC * CH], FP, tag="l", name=f"l_{c}")
                ad_sb = nsb.tile([P, CH], FP, tag="ad")
                u_sb = nsb.tile([P, CH], FP, tag="u")
                bx_sb = nsb.tile([P, CH], FP, tag="bx")
                # softplus(g+b) = ln(exp(g+b) + 1); Exp/Ln/Copy share one
                # ACT table (natural_log_exp_and_others) -> no table thrash
                nc.scalar.activation(
                    out=e_sb, in_=psd,
                    func=mybir.ActivationFunctionType.Exp,
                    bias=pcols_sb[:, 0:1])
                nc.scalar.activation(
                    out=l_sb, in_=e_sb,
                    func=mybir.ActivationFunctionType.Ln,
                    bias=1.0)
                # Ad batch-pack: rows (b*64) <- exp(A * softplus_del(b))
                for b in range(BPC):
                    nc.scalar.activation(
                        out=ad_sb[N * b:N * (b + 1), :],
                        in_=l_sb[N:P, b * CH:(b + 1) * CH],
                        func=mybir.ActivationFunctionType.Exp,
                        scale=pcols_sb[N:P, 2:3])
                # u = (P + pbias) * sel, batch-packed rows
                for b in range(BPC):
                    nc.vector.scalar_tensor_tensor(
                        out=u_sb[N * b:N * (b + 1), :],
                        in0=pP[N * b:N * (b + 1), :],
                        scalar=pcols_sb[0:N, 3:4],
                        in1=l_sb[0:N, b * CH:(b + 1) * CH],
                        op0=AOP.add, op1=AOP.mult)
                # bx = (Ad - 1) * u
                nc.vector.scalar_tensor_tensor(
                    out=bx_sb, in0=ad_sb, scalar=-1.0, in1=u_sb,
                    op0=AOP.add, op1=AOP.mult)
                s_prev = s_tile
                s_tile = scanp.tile([P, CH], FP, tag="s")
                nc.vector.tensor_tensor_scan(
                    out=s_tile, data0=ad_sb, data1=bx_sb,
                    initial=(0.0 if c == 1 else s_prev[:, CH - 1:CH]),
                    op0=AOP.mult, op1=AOP.add)

        # tail: y = s_last @ blockdiag(CmT*invA) + conv(x)[L-1] @ DmT + ybias
        py = psum.tile([1, 2 * O], FP, tag="bm", bufs=1)
        nc.tensor.matmul(out=py, lhsT=s_tile[:, CH - 1:CH], rhs=cmT_sb,
                         start=True, stop=False, skip_group_check=True)
        for b in range(BPC):
            for h in (0, 1):
                for k in (0, 1):  # taps 0,1 of xc[L-1]; tap 2 is x[L] = 0
                    nc.tensor.matmul(
                        out=py[0:1, O * b:O * (b + 1)],
                        lhsT=xts[h][:, b, L - 1 + k:L + k],
                        rhs=dmT_sb[:, h, k, :],
                        start=False, stop=(b == 1 and h == 1 and k == 1),
                        skip_group_check=True)
        y_sb = consts.tile([1, 2 * O], FP, tag="ysb")
        nc.vector.tensor_add(y_sb, py, ybias_sb)
        nc.sync.dma_start(out=y, in_=y_sb)

    nc.compile()
    return nc


def _prep_params(sel_W, sel_b, selection_bias, A_log, Bm, Cm, Dm,
                 delta_W, delta_b, conv_w, conv_b):
    f = np.float32
    h16 = np.float16
    sel_W = np.asarray(sel_W, f)
    delta_W = np.asarray(delta_W, f)
    Bm = np.asarray(Bm, f)
    Cm = np.asarray(Cm, f)
    Dm = np.asarray(Dm, f)
    conv_w = np.asarray(conv_w, f)      # [D, 1, 3]
    conv_b = np.asarray(conv_b, f)
    sel_b = np.asarray(sel_b, f)
    selection_bias = np.asarray(selection_bias, f)
    delta_b = np.asarray(delta_b, f)
    A_log = np.asarray(A_log, f)

    A = -np.exp(A_log.astype(np.float64))
    invA = 1.0 / (A + 1e-8)
    cw = conv_w[:, 0, :]                # [D, 3]

    # lhsT with conv tap folded: wk[kd, h, tap, j] = W[j, h*128+kd] * cw[.,tap]
    Wcat = np.concatenate([sel_W, delta_W, Bm], axis=0)   # [192, D]
    wk = np.zeros((P, 2, 3, 3 * N), f)
    for h in (0, 1):
        for k in (0, 1, 2):
            Wf = Wcat * cw[None, :, k]
            wk[:, h, k, :] = Wf[:, h * P:(h + 1) * P].T

    bias_sel = sel_b + selection_bias + sel_W @ conv_b
    bias_del = delta_b + delta_W @ conv_b
    pbias = Bm @ conv_b
    pcols = np.zeros((P, 4), f)
    pcols[:, 0] = np.concatenate([bias_sel, bias_del])
    pcols[:, 2] = np.tile(A.astype(f), 2)
    pcols[:, 3] = np.tile(pbias, 2)

    cmT = np.zeros((P, 2 * O), f)
    blk = (Cm.T.astype(np.float64) * invA[:, None]).astype(f)  # [N, O]
    cmT[0:N, 0:O] = blk
    cmT[N:2 * N, O:2 * O] = blk

    dmT = np.zeros((P, 2, 2, O), f)
    for h in (0, 1):
        for k in (0, 1):
            Df = Dm * cw[None, :, k]
            dmT[:, h, k, :] = Df[:, h * P:(h + 1) * P].T

    ybias = np.tile(Dm @ conv_b, 2)[None, :].astype(f)
    identity = np.eye(P, dtype=f)

    return dict(wk=np.ascontiguousarray(wk).astype(h16), pcols=pcols,
                cmT=cmT, dmT=np.ascontiguousarray(dmT).astype(h16),
                ybias=ybias, ident=identity.astype(h16))


_CACHED = {}


def _get_program():
    if "nc" not in _CACHED:
        _CACHED["nc"] = _build_program()
    return _CACHED["nc"]


def kernel(x, sel_W, sel_b, selection_bias, A_log, Bm, Cm, Dm,
           delta_W, delta_b, conv_w, conv_b, _trace=False):
    x = np.ascontiguousarray(np.asarray(x, np.float32))
    params = _prep_params(sel_W, sel_b, selection_bias, A_log, Bm, Cm, Dm,
                          delta_W, delta_b, conv_w, conv_b)
    nc = _get_program()
    in_maps = []
    for c in range(NCORES):
        m = dict(params)
        m["xs"] = np.ascontiguousarray(x[BPC * c:BPC * (c + 1)])
        in_maps.append(m)
    res = run_bass_kernel_spmd(nc, in_maps, core_ids=list(range(NCORES)),
                               trace=_trace)
    out = np.concatenate(
        [res.results[c]["y"].reshape(BPC, O) for c in range(NCORES)], axis=0)
    if _trace:
        _CACHED["last_results"] = res
    return out
